# revision 27
# baseline (speedup 1.0000x reference)
"""Multi-head causal attention (b=4, l=2048, d=1024, 16 heads x 64) on 8 trn2 cores.

Sharding: core c handles batch (c // 2) and head-group (c % 2) of 8 heads.
Each core computes a partial output x[b] @ W (its 8 heads' contribution);
the host sums the two partials per batch.

V2 design (sim 183.9us vs the 215us bf16 baseline; HW-validated,
relative error 1.19e-2 against the f32 reference, gate 2e-2):
  - QKV projection in fp8e4 DoubleRow matmuls, 3-plane error-compensated:
    x and W are split hi/lo on the HOST (x*16, W*128 -> e4m3 hi + e4m3
    residual lo; the power-of-2 scales keep everything in e4m3's normal
    range).  Per d-pair the planes (Wh,Wh')x(xh,xh'), (Wh,Wh')x(xl,xl'),
    (Wl,Wl')x(xh,xh') pack two 128-K chunks per DoubleRow at 0.5 cy/row
    => QKV PE time 82us -> 61.5us at ~3e-3 end-to-end error (better
    than bf16).  Descale 1/2048 folds into the psum evacuations.
  - S = K^T Q in fp8 DoubleRow, 2-plane: qT stored as (hi, lo) e4m3,
    kT single e4m3 broadcast to both weight planes (0-step AP, probed
    on HW); S psum = (16k)^T(16qh+16ql) = 256*S exactly => exp scale
    1/2048.  0.5 cy/row => S PE time 61.5us -> 30.7us at ~1.2e-2
    end-to-end (the k-side single-fp8 rounding dominates the budget).
  - PV and out-projection stay bf16: P, V, or A in fp8 each measured
    >2.8e-2 end-to-end (softmax weights/values are error-critical).
  - exp runs on TWO engines: z=0 heads' full pairs on Act ([128,2,512]
    activation, scale=1/2048); z=1 heads' full pairs as 2^(S*log2e/2048)
    via DVE scale-mul (psum->sbuf f32, gpsimd cannot read PSUM - BIR
    verifier) + gpsimd pow with a broadcast base-2 tile (~4e-3 rel,
    probed).  Diagonal pairs stay on Act (trim + affine_select).
  - the two z-heads of each head-pair are interleaved round-by-round
    (one S/exp round per pair index u) so Act and DVE+Pool exp chains
    run concurrently; PV lags 2 rounds on the Act stream and 3 on the
    longer pool stream; each pair's PV-tail/normalize is deferred into
    the next head-pair's first round to fill the boundary bubble.
  - backfill: chunk j+1's QKV DoubleRows (tagged per head-pair cc
    deadline) fill j<=2's exp waits; all out-projections fill the
    chain-bound j=3; q-hi/k evacuations on Act, q-lo/v on DVE.
  - PSUM: psS 2x[128,2,512] + psO 2x[128,4,65] (lazy-allocated at first
    PV) + psW 2x[128,512] = 8 banks.  j=3 is bound by the two psS-slot
    exp chains (exp 1.04us + sem/S/sem ~0.6us per round and slot); more
    chain parallelism needs psum banks that aren't there.
  - everything else (PV ones-column sums, xbar transpose of O, row-0
    host patch) as the baseline.
"""

import sys

sys.path.insert(0, "/opt/trn_rl_repo")

import numpy as np
import ml_dtypes

import concourse.bacc as bacc
import concourse.mybir as mybir
import concourse.tile as tile
from concourse.bass_utils import run_bass_kernel_spmd

F32 = mybir.dt.float32
BF16 = mybir.dt.bfloat16
F8 = mybir.dt.float8e4
AF = mybir.ActivationFunctionType
ALU = mybir.AluOpType
DR = mybir.MatmulPerfMode.DoubleRow

B, L, D = 4, 2048, 1024
N_HEAD, KEY_DIM = 16, 64
HG = 8               # heads per core (head-group)
C = HG * KEY_DIM     # 512 per-core qkv width
NLC = 16             # l chunks of 128
NJ = 4               # l/q chunks of 512
ND = 8               # d chunks of 128
NDP = 4              # d chunk pairs
NCC = 4              # c chunks of 128

SX = 16.0            # x fp8 pre-scale (host)
SW = 128.0           # W fp8 pre-scale (host)
SQK = 16.0           # q,k fp8 scale on chip
EXP_SCALE = 1.0 / (8.0 * SQK * SQK)       # softmax scale on S psum (=1/2048)
DESCALE_QK = SQK / (SX * SW)              # qkv psum -> 16*q fp8   (=1/128)
DESCALE_V = 1.0 / (SX * SW)               # qkv psum -> v bf16     (=1/2048)
POW_SCALE = float(np.log2(np.e)) * EXP_SCALE  # S psum -> exponent of 2
POP_BUDGET = [2.0, 1.0, 0.7, 0.5]  # us of filler PE-work popped per round

_CACHED = {}


def _pool_exp_pairs(j):
    """Full (non-diagonal) pairs of q-chunk j whose z=1 exp runs on
    DVE+gpsimd instead of Act, sized to each phase's Act/DVE/PE slack."""
    return set(range(2 * j))


def build_nc():
    nc = bacc.Bacc("TRN2", target_bir_lowering=False, debug=False)

    # x/W hi-lo fp8 planes, packed per d-pair: [dp][128 part][d-in-pair][hl][...]
    xhl = nc.dram_tensor("xhl", [NDP, 128, 2, 2, L], F8, kind="ExternalInput")
    wq = nc.dram_tensor("wq", [NDP, 128, 2, 2, C], F8, kind="ExternalInput")
    wk = nc.dram_tensor("wk", [NDP, 128, 2, 2, C], F8, kind="ExternalInput")
    wv = nc.dram_tensor("wv", [NDP, 128, 2, 2, C], F8, kind="ExternalInput")
    wo = nc.dram_tensor("wo", [C, D], BF16, kind="ExternalInput")
    out = nc.dram_tensor("out", [L, D], BF16, kind="ExternalOutput")

    with tile.TileContext(nc) as tc:
        with tc.tile_pool(name="persist", bufs=1) as persist, \
             tc.tile_pool(name="wpool", bufs=1) as wpool, \
             tc.tile_pool(name="xt", bufs=8) as xtp, \
             tc.tile_pool(name="pp", bufs=12) as pp, \
             tc.tile_pool(name="sf", bufs=4) as sfp, \
             tc.tile_pool(name="ofnp", bufs=3) as ofnp, \
             tc.tile_pool(name="rp", bufs=4) as rp, \
             tc.tile_pool(name="osb", bufs=3) as osb, \
             tc.tile_pool(name="psS", bufs=2, space="PSUM") as psS, \
             tc.tile_pool(name="psO", bufs=2, space="PSUM") as psO, \
             tc.tile_pool(name="psW", bufs=2, space="PSUM") as psW:

            # per-chunk persistent tensors (chunked to keep deps precise)
            qT = [[persist.tile([128, 2, 512], F8, name=f"qT{lc}_{t}")
                   for t in range(NCC)] for lc in range(NJ)]
            kT = [[persist.tile([128, 1, 512], F8, name=f"kT{lc}_{t}")
                   for t in range(NCC)] for lc in range(NJ)]
            vp = [persist.tile([128, HG, KEY_DIM + 1], BF16, name=f"vp{i}")
                  for i in range(NLC)]
            # OFT[j][t]: normalized attention output, c-major (out-proj lhsT)
            OFT = [[persist.tile([128, 512], BF16, name=f"oft{j}_{t}")
                    for t in range(NCC)] for j in range(NJ)]

            wq_sb = [wpool.tile([128, 2, 2, C], F8, name=f"wq{d}") for d in range(NDP)]
            wk_sb = [wpool.tile([128, 2, 2, C], F8, name=f"wk{d}") for d in range(NDP)]
            wv_sb = [wpool.tile([128, 2, 2, C], F8, name=f"wv{d}") for d in range(NDP)]
            wo_sb = [wpool.tile([128, D], BF16, name=f"wo{t}") for t in range(NCC)]
            base2 = wpool.tile([128, 1, 1], BF16, name="base2")
            nc.vector.memset(base2[:], 2.0)

            def dma_x(lc):
                ls = slice(512 * lc, 512 * (lc + 1))
                xts = []
                for dp in range(NDP):
                    t = xtp.tile([128, 2, 2, 512], F8, name=f"xt{lc}_{dp}",
                                 tag="xt")
                    nc.sync.dma_start(t[:], xhl[dp, :, :, :, ls])
                    xts.append(t)
                return xts

            # DMA order matters: all transfers serialize on the DMA engines,
            # so load exactly what the first projection groups need first —
            # wq/x interleaved per d-pair so matmuls on pair dp can start
            # while dp+1 is still in flight.
            xts0 = []
            for dp in range(NDP):
                nc.sync.dma_start(wq_sb[dp][:], wq[dp, :, :, :, :])
                t = xtp.tile([128, 2, 2, 512], F8, name=f"xt0_{dp}", tag="xt")
                nc.sync.dma_start(t[:], xhl[dp, :, :, :, 0:512])
                xts0.append(t)
            for dp in range(NDP):
                nc.sync.dma_start(wk_sb[dp][:], wk[dp, :, :, :, :])
            for dp in range(NDP):
                nc.sync.dma_start(wv_sb[dp][:], wv[dp, :, :, :, :])

            for i in range(NLC):
                # whole-tile memset; v evacuation overwrites cols 0..63 of
                # each head slot, col 64 stays 1.0 (the softmax-sum row)
                nc.gpsimd.memset(vp[i][:], 1.0)

            def qk_drs(ps, w_sb, xts, dp, cc, first, last):
                """3 DoubleRow matmuls: c-major q/k partial for d-pair dp."""
                ccs = slice(128 * cc, 128 * (cc + 1))
                w, x = w_sb[dp], xts[dp]
                nc.tensor.matmul(ps[:], w[:, :, 0, ccs], x[:, :, 0, :],
                                 start=first, stop=False, perf_mode=DR)
                nc.tensor.matmul(ps[:], w[:, :, 0, ccs], x[:, :, 1, :],
                                 start=False, stop=False, perf_mode=DR)
                nc.tensor.matmul(ps[:], w[:, :, 1, ccs], x[:, :, 0, :],
                                 start=False, stop=last, perf_mode=DR)

            def v_drs(ps, xts, dp, lcc, first, last):
                """3 DoubleRow matmuls: l-major v partial for d-pair dp."""
                ls = slice(128 * lcc, 128 * (lcc + 1))
                x, w = xts[dp], wv_sb[dp]
                nc.tensor.matmul(ps[:], x[:, :, 0, ls], w[:, :, 0, :],
                                 start=first, stop=False, perf_mode=DR)
                nc.tensor.matmul(ps[:], x[:, :, 1, ls], w[:, :, 0, :],
                                 start=False, stop=False, perf_mode=DR)
                nc.tensor.matmul(ps[:], x[:, :, 0, ls], w[:, :, 1, :],
                                 start=False, stop=last, perf_mode=DR)

            def evac_qk(dst_hl, ps, lo, on_act=False):
                if not lo:
                    if on_act:
                        # chunk 0/1 q-hi on Act (idle during the lead-in)
                        nc.scalar.activation(dst_hl[:, 0, :], ps[:], AF.Copy,
                                             scale=DESCALE_QK)
                    else:
                        nc.vector.tensor_scalar_mul(dst_hl[:, 0, :], ps[:],
                                                    DESCALE_QK)
                else:
                    nc.vector.scalar_tensor_tensor(
                        dst_hl[:, 1, :], ps[:], DESCALE_QK, dst_hl[:, 0, :],
                        ALU.mult, ALU.subtract)

            def proj_groups(lc, xts):
                """Return (tag, closure) pairs of 3 DoubleRows each; tags
                encode the deadline: v by attention-lc's first PV, q/k
                cc-group by head-pair hp=cc of attention chunk lc."""
                groups = []
                state = {}
                for lcc in range(4):
                    for dp in range(NDP):
                        def g(lcc=lcc, dp=dp):
                            i = 4 * lc + lcc
                            if dp == 0:
                                state["ps"] = psW.tile(
                                    [128, 512], F32, name=f"pv{i}", tag="psW")
                            ps = state["ps"]
                            v_drs(ps, xts, dp, lcc, dp == 0, dp == NDP - 1)
                            if dp == NDP - 1:
                                # Pool/GPSIMD cannot read PSUM on real HW
                                nc.vector.tensor_scalar_mul(
                                    vp[i][:, :, 0:KEY_DIM],
                                    ps[:].rearrange("p (h c) -> p h c", h=HG),
                                    DESCALE_V)
                        groups.append((lc + 0.0005, 0.32, g))
                for cc in range(NCC):
                    for qk, (w_sb, dst) in enumerate(((wq_sb, qT), (wk_sb, kT))):
                        for dp in range(NDP):
                            def g(qk=qk, w_sb=w_sb, dst=dst, cc=cc, dp=dp):
                                if dp == 0:
                                    state["ps"] = psW.tile(
                                        [128, 512], F32,
                                        name=f"pj{lc}_{qk}{cc}", tag="psW")
                                ps = state["ps"]
                                qk_drs(ps, w_sb, xts, dp, cc,
                                       dp == 0, dp == NDP - 1)
                                if dp == NDP - 1:
                                    on_act = True
                                    if qk == 0:
                                        evac_qk(dst[lc][cc], ps, lo=False,
                                                on_act=on_act)
                                        evac_qk(dst[lc][cc], ps, lo=True)
                                    else:
                                        if on_act:
                                            nc.scalar.activation(
                                                dst[lc][cc][:, 0, :], ps[:],
                                                AF.Copy, scale=DESCALE_QK)
                                        else:
                                            nc.vector.tensor_scalar_mul(
                                                dst[lc][cc][:, 0, :], ps[:],
                                                DESCALE_QK)
                            groups.append((lc + 0.1 * cc - 0.002, 0.32, g))
                return groups

            def pv_pair2(o_ps, p_sb, h, u, n_i, state, stop):
                """P^T V for pair u: out [128 q, 65] per 128-q sub-chunk,
                col 64 accumulates the softmax sums via vp's ones column.
                All pairs accumulate into one PSUM group in emission order:
                start on the first matmul emitted, stop on the last."""
                j = (n_i // 4) - 1
                for w in range(2):
                    i = 2 * u + w
                    for qs in range(4):
                        if u >= 2 * j and qs < 2 * (u - 2 * j) + w:
                            continue  # q-block fully below the causal mask
                        nc.tensor.matmul(
                            o_ps[:, qs, :],
                            p_sb[:, w, 128 * qs:128 * (qs + 1)],
                            vp[i][:, h, :],
                            start=not state["started"],
                            stop=(stop and w == 1 and qs == 3),
                            skip_group_check=True)
                        state["started"] = True

            def attn_head_pair(j, hp, filler, pops, pending_tail):
                """Attention for heads (2hp, 2hp+1), q-chunk j, with the two
                z-heads interleaved round-by-round: z0's exp runs on Act,
                z1's on DVE+gpsimd for pairs in the offload set, so each
                round feeds two exp engines while PE runs S/PV/filler.
                The PV tail + normalize of the PREVIOUS head-pair arrives
                as `pending_tail` and is emitted after this pair's first
                round, filling the boundary pipeline bubble; this pair's
                tail is returned as a closure for the same treatment."""
                nu = 2 * (j + 1)
                n_i = 4 * (j + 1)
                pool_us = _pool_exp_pairs(j)
                o_ps = [None, None]
                p_tiles = {}
                pv_state = [{"started": False}, {"started": False}]

                def pv_emit(u, z, stop=False):
                    # the v-projection of this chunk must be emitted before
                    # the first PV reads vp (tags lc + 0.0005)
                    while filler and filler[0][0] <= j + 0.001:
                        filler.pop(0)[2]()
                    if o_ps[z] is None:
                        o_ps[z] = psO.tile([128, 4, 65], F32,
                                           name=f"o{j}{hp}{z}", tag="psO")
                    pv_pair2(o_ps[z], p_tiles[(u, z)], 2 * hp + z, u, n_i,
                             pv_state[z], stop)

                for u in range(nu):
                    for z in range(2):
                        rows = slice(64 * z, 64 * z + 64)
                        s_ps = psS.tile([128, 2, 512], F32,
                                        name=f"s{j}{hp}{z}{u}", tag="psS")
                        for w in range(2):
                            i = 2 * u + w
                            # diagonal pairs: columns left of 256*w0 are dead
                            # (skipped by pv_pair / zeroed by select) and not
                            # read by the trimmed exp, so don't compute them
                            st = 256 * (u - 2 * j) if u >= 2 * j else 0
                            kap = kT[i // 4][hp][rows, :,
                                                 128 * (i % 4):128 * (i % 4 + 1)]
                            nc.tensor.matmul(
                                s_ps[:, w, st:512],
                                kap.broadcast_to([64, 2, 128]),
                                qT[j][hp][rows, :, st:512],
                                start=True, stop=True, perf_mode=DR)
                        p_sb = pp.tile([128, 2, 512], BF16,
                                       name=f"p{j}{hp}{z}{u}", tag="pp")
                        if u < 2 * j:
                            if z == 1 and u in pool_us:
                                # exp on DVE+gpsimd: p = 2^(S * log2e/2048)
                                sf = sfp.tile([128, 2, 512], F32,
                                              name=f"sf{j}{hp}{u}", tag="sf")
                                nc.vector.tensor_scalar_mul(sf[:], s_ps[:],
                                                            POW_SCALE)
                                nc.gpsimd.tensor_tensor(
                                    p_sb[:],
                                    base2[:].broadcast_to([128, 2, 512]),
                                    sf[:], ALU.pow)
                            else:
                                nc.scalar.activation(p_sb[:], s_ps[:], AF.Exp,
                                                     scale=EXP_SCALE)
                        else:
                            # diagonal band pair w0: columns below 256*w0 are
                            # fully masked AND never read by pv_pair - exp
                            # the live range, then select the partial strip
                            w0 = u - 2 * j  # 0 or 1
                            cs = slice(256 * w0, 512)
                            nc.scalar.activation(p_sb[:, :, cs],
                                                 s_ps[:, :, cs],
                                                 AF.Exp, scale=EXP_SCALE)
                            sel = slice(256 * w0, 256 * w0 + 256)
                            nc.gpsimd.affine_select(
                                out=p_sb[:, :, sel], in_=p_sb[:, :, sel],
                                compare_op=ALU.is_gt, fill=0.0,
                                base=0, channel_multiplier=-1,
                                pattern=[[-128, 2], [1, 256]])
                        p_tiles[(u, z)] = p_sb
                    acc = 0.0
                    while filler and acc < pops:
                        _, cost, g = filler.pop(0)
                        g()
                        acc += cost
                    # progressive deadline: spread tagged filler (incl. the
                    # ph3 out-projections) evenly across this pair's rounds
                    key = j + 0.1 * hp + 0.1 * (u + 1) / nu
                    while filler and filler[0][0] <= key:
                        filler.pop(0)[2]()
                    if u == 0 and pending_tail is not None:
                        pending_tail()
                        pending_tail = None
                    # lag PV 2 rounds behind the exp pipeline on Act's
                    # z0 stream, 3 on the longer DVE->Pool z1 stream
                    if u >= 2:
                        pv_emit(u - 2, 0)
                    if u >= 3:
                        pv_emit(u - 3, 1)
                if pending_tail is not None:
                    pending_tail()

                def tail():
                    if nu >= 2:
                        pv_emit(nu - 2, 0)
                        if nu >= 3:
                            pv_emit(nu - 3, 1)
                        pv_emit(nu - 2, 1)
                    pv_emit(nu - 1, 0, stop=True)
                    pv_emit(nu - 1, 1, stop=True)
                    # normalize straight out of PSUM with per-partition
                    # scalars; the two z-heads share one q-major tile (z=0
                    # fills c-columns 0:64, z=1 fills 64:128) so the xbar
                    # transpose below emits full 128-col tiles
                    ofn = ofnp.tile([128, 4, 128], BF16, name=f"ofn{j}{hp}",
                                    tag="ofn")
                    for z in range(2):
                        r_sb = rp.tile([128, 4, 1], F32, name=f"r{j}{hp}{z}",
                                       tag="rp")
                        nc.vector.reciprocal(r_sb[:], o_ps[z][:, :, 64:65])
                        nc.vector.tensor_tensor(
                            ofn[:, :, 64 * z:64 * z + 64],
                            o_ps[z][:, :, 0:64],
                            r_sb[:].broadcast_to([128, 4, 64]), ALU.mult)
                    # transpose O back to c-major via the (idle) DMA xbar
                    for qs in range(4):
                        nc.sync.dma_start_transpose(
                            OFT[j][hp][:, 128 * qs:128 * (qs + 1)],
                            ofn[:, qs, :])
                return tail

            ostate = {}

            def out_proj_closures(j, act_evac=False):
                cls = []
                for qc in range(4):
                    for n in range(2):
                        for t_ in range(4):
                            def g(j=j, qc=qc, n=n, t_=t_):
                                qs = slice(128 * qc, 128 * (qc + 1))
                                ns = slice(512 * n, 512 * (n + 1))
                                if t_ == 0:
                                    ostate["ps"] = psW.tile(
                                        [128, 512], F32,
                                        name=f"f{j}{qc}{n}", tag="psW")
                                f_ps = ostate["ps"]
                                nc.tensor.matmul(
                                    f_ps[:], OFT[j][t_][:, qs],
                                    wo_sb[t_][:, ns],
                                    start=(t_ == 0), stop=(t_ == NCC - 1))
                                if t_ == 3:
                                    o_sb = osb.tile([128, 512], BF16,
                                                    name=f"ob{j}{qc}{n}",
                                                    tag="osb")
                                    if act_evac == "split" and (qc + n) % 2 or act_evac is True:
                                        nc.scalar.copy(o_sb[:], f_ps[:])
                                    else:
                                        nc.vector.tensor_scalar_mul(
                                            o_sb[:], f_ps[:], 1.0)
                                    lo = 512 * j + 128 * qc
                                    nc.sync.dma_start(out[lo:lo + 128, ns],
                                                      o_sb[:])
                            cls.append(g)
                return cls

            # ---- main software-pipelined loop ----
            # j=0..2 attention is backfilled with the next chunk's QKV
            # projection; the Act-bound j=3 is backfilled with ALL the
            # deferred output projections of j=0..2.
            # lead-in projection is DMA-paced: emit d-pair-major across four
            # concurrent psum groups so each arriving w/x d-pair tile
            # immediately feeds 12 matmuls (psS slots are idle; borrow two)
            def proj0_lead(xts):
                """Emit q/k for cc=0 (dp-major, 2 concurrent groups) then v
                (dp-major, 4 groups). cc1-3 of q/k are returned as tagged
                filler closures so attention j=0 can start immediately."""
                ps2 = [psW.tile([128, 512], F32, name=f"p0qk{w}", tag="psW")
                       for w in range(2)]
                for which, (w_sb, dst) in enumerate(((wq_sb, qT), (wk_sb, kT))):
                    for dp in range(NDP):
                        qk_drs_ap(ps2[which][:], w_sb, xts, dp, 0,
                                  dp == 0, dp == NDP - 1)
                    if which == 0:
                        evac_qk_ap(dst[0][0], ps2[which][:], lo=False)
                        evac_qk_ap(dst[0][0], ps2[which][:], lo=True)
                    else:
                        nc.scalar.activation(dst[0][0][:, 0, :], ps2[which][:],
                                             AF.Copy, scale=DESCALE_QK)
                # remaining chunk-0 q/k and all of v as filler: q/k
                # cc-groups keyed to head-pair hp of j=0, v keyed to the
                # first PV of j=0 (so the first S isn't held behind v)
                groups = []
                state = {}
                for lcc in range(4):
                    for dp in range(NDP):
                        def g(lcc=lcc, dp=dp):
                            if dp == 0:
                                state["ps"] = psW.tile(
                                    [128, 512], F32, name=f"p0v{lcc}",
                                    tag="psW")
                            ps = state["ps"]
                            v_drs(ps, xts, dp, lcc, dp == 0, dp == NDP - 1)
                            if dp == NDP - 1:
                                nc.vector.tensor_scalar_mul(
                                    vp[lcc][:, :, 0:KEY_DIM],
                                    ps[:].rearrange("p (h c) -> p h c", h=HG),
                                    DESCALE_V)
                        groups.append((0.0005, 0.32, g))
                for cc in range(1, NCC):
                    for which, (w_sb, dst) in enumerate(((wq_sb, qT),
                                                        (wk_sb, kT))):
                        for dp in range(NDP):
                            def g(which=which, w_sb=w_sb, dst=dst, cc=cc,
                                  dp=dp):
                                if dp == 0:
                                    state["ps"] = psW.tile(
                                        [128, 512], F32,
                                        name=f"p0f{which}{cc}", tag="psW")
                                ps = state["ps"]
                                qk_drs_ap(ps[:], w_sb, xts, dp, cc,
                                          dp == 0, dp == NDP - 1)
                                if dp == NDP - 1:
                                    if which == 0:
                                        evac_qk_ap(dst[0][cc], ps[:], lo=False)
                                        evac_qk_ap(dst[0][cc], ps[:], lo=True)
                                    else:
                                        nc.scalar.activation(
                                            dst[0][cc][:, 0, :], ps[:],
                                            AF.Copy, scale=DESCALE_QK)
                            groups.append((0.1 * cc - 0.001, 0.32, g))
                return groups

            def qk_drs_ap(ps_ap, w_sb, xts, dp, cc, first, last):
                ccs = slice(128 * cc, 128 * (cc + 1))
                w, x = w_sb[dp], xts[dp]
                nc.tensor.matmul(ps_ap, w[:, :, 0, ccs], x[:, :, 0, :],
                                 start=first, stop=False, perf_mode=DR)
                nc.tensor.matmul(ps_ap, w[:, :, 0, ccs], x[:, :, 1, :],
                                 start=False, stop=False, perf_mode=DR)
                nc.tensor.matmul(ps_ap, w[:, :, 1, ccs], x[:, :, 0, :],
                                 start=False, stop=last, perf_mode=DR)

            def evac_qk_ap(dst_hl, ps_ap, lo):
                if not lo:
                    nc.scalar.activation(dst_hl[:, 0, :], ps_ap, AF.Copy,
                                         scale=DESCALE_QK)
                else:
                    nc.vector.scalar_tensor_tensor(
                        dst_hl[:, 1, :], ps_ap, DESCALE_QK, dst_hl[:, 0, :],
                        ALU.mult, ALU.subtract)

            sched = [(j, hp) for j in range(NJ) for hp in range(4)]

            filler = list(proj0_lead(xts0))
            pending_tail = None
            started = set()
            done_count = [0] * NJ
            rounds_left = sum(2 * (jj + 1) for jj, _ in sched)
            for j, hp in sched:
                if j not in started:
                    started.add(j)
                    if j == 1:
                        # wo is only read by ph3's out-projections; keep it
                        # behind chunk-1's x in the lead DMA queue
                        for t_ in range(NCC):
                            nc.sync.dma_start(wo_sb[t_][:],
                                              wo[128 * t_:128 * (t_ + 1), :])
                    if j + 1 < NJ:
                        xts = dma_x(j + 1)
                        filler.extend(proj_groups(j + 1, xts))
                    if j == NJ - 1:
                        # all deferred out-projections backfill the
                        # exp-bound final chunk, deadline-spread over it
                        cls = [g for jj in range(NJ - 1)
                               for g in out_proj_closures(jj, "split")]
                        filler.extend((j + 0.02 + 0.36 * idx / len(cls),
                                       0.21, g)
                                      for idx, g in enumerate(cls))
                # hard guard: everything attention (j, hp) reads must be
                # emitted before its first S matmul; chunk cc-groups carry
                # fractional keys of the head-pair that needs them
                while filler and filler[0][0] <= j + 0.1 * hp:
                    filler.pop(0)[2]()
                pending_tail = attn_head_pair(j, hp, filler,
                                              POP_BUDGET[j], pending_tail)
                rounds_left -= 2 * (j + 1)
                done_count[j] += 1
            pending_tail()
            # drain remaining filler + last chunk's out-projection
            for _, _, g in filler:
                g()
            # tail: DVE is idle after the last normalize, evacuate there
            for g in out_proj_closures(NJ - 1, act_evac=None):
                g()

    nc.finalize()
    return nc


def _get_nc():
    if "nc" not in _CACHED:
        _CACHED["nc"] = build_nc()
    return _CACHED["nc"]


def _hi_lo_pack(a, scale):
    """a [d, n] f32 -> [NDP, 128, 2, 2, n] e4m3 (d-pair, part, d, hl, n)."""
    f8 = ml_dtypes.float8_e4m3
    hi = (a * scale).astype(f8)
    lo = (a * scale - hi.astype(np.float32)).astype(f8)
    d, n = a.shape
    st = np.stack([hi, lo], axis=1).reshape(ND, 128, 2, n)   # [8,128,2,n]
    st = st.reshape(NDP, 2, 128, 2, n).transpose(0, 2, 1, 3, 4)
    return np.ascontiguousarray(st)


def kernel(x, W_q, W_k, W_v, W_out, trace=False, trace_kwargs=None):
    x = np.asarray(x, dtype=np.float32)
    W_q = np.asarray(W_q, dtype=np.float32)
    W_k = np.asarray(W_k, dtype=np.float32)
    W_v = np.asarray(W_v, dtype=np.float32)
    W_out = np.asarray(W_out, dtype=np.float32)
    bf = ml_dtypes.bfloat16

    nc = _get_nc()
    in_maps = []
    for core in range(8):
        b, g = core // 2, core % 2
        cs = slice(C * g, C * (g + 1))
        in_maps.append({
            "xhl": _hi_lo_pack(x[b].T, SX),
            "wq": _hi_lo_pack(W_q[:, cs], SW),
            "wk": _hi_lo_pack(W_k[:, cs], SW),
            "wv": _hi_lo_pack(W_v[:, cs], SW),
            "wo": np.ascontiguousarray(W_out[cs, :]).astype(bf),
        })
    res = run_bass_kernel_spmd(nc, in_maps, core_ids=list(range(8)),
                               trace=trace, **(trace_kwargs or {}))
    out = np.empty((B, L, D), dtype=np.float32)
    for b in range(B):
        out[b] = (res.results[2 * b]["out"].astype(np.float32)
                  + res.results[2 * b + 1]["out"].astype(np.float32))
        # q=0 is fully masked -> reference softmax gives uniform attention over
        # all of V; the device leaves NaN/0 in that row, patch it here.
        out[b, 0, :] = (x[b].mean(axis=0) @ W_v) @ W_out
    if trace:
        return out, res
    return out
